# revision 1
# baseline (speedup 1.0000x reference)
"""DocRE (ATLOP-style) Trainium2 kernel, v3.

Sharding: data-parallel over the 4 documents x 2-way split of the entity-row
axis (i) -> 8 cores.  Each core computes the full pipeline for its 16x32
entity-pair block; cross-core coupling (column softmax + column context) is
handled with two small pair collectives (AllReduce of the local column sums
of exp(e) [128B], AllReduce of the unnormalized classified column context
[12KB]); softmax normalizations are folded in after the collectives.

Key structure:
- pa row-sums come out of the ctx matmul via a ones-column appended to the
  SWc rhs, directly in pair-partition layout (no DRAM roundtrip); all 4
  pair-chunk ctx accumulators chase the pa tiles in PSUM.
- z = tanh((ctx + rsum*repW) / rsum): one DVE scalar_tensor_tensor with the
  PSUM row-sum as the scalar, then a scalar-engine tanh with scale=1/rsum —
  no extra scale pass; z/T work interleaves so DVE never waits on Act.
- the bilinear g is never materialized against AfT: AW = Af@W_attn and
  ACl = Af@clf_W are pre-contracted on device during the DMA shadow, so the
  post-T chain is T^T -> th/e and T^T -> per-pair classifier logits directly
  (everything bf16, pair-major), with tree tails on the GPSIMD engine.
- row/col softmax contexts are taken through the classifier as
  Mrc^T @ (T ACl) [48, C], normalized post-collective, and scattered back to
  pairs with transposed-selector matmuls; output is written pair-major.
"""

import os
import sys
from contextlib import ExitStack

import numpy as np

for _p in ("/opt/trn_rl_repo", "/root/.axon_site/_ro/trn_rl_repo"):
    if os.path.isdir(_p) and _p not in sys.path:
        sys.path.insert(0, _p)

import ml_dtypes  # noqa: E402
import concourse.bass as bass  # noqa: E402,F401
import concourse.tile as tile  # noqa: E402
from concourse import bacc, mybir  # noqa: E402
from concourse.bass_utils import run_bass_kernel_spmd  # noqa: E402

# problem dims (hardcoded per spec)
B, S, H, HH, N, M = 4, 1024, 768, 12, 32, 4
Wd, AD, C, KK = 768, 256, 97, 64
DD = Wd // KK          # 12
NI = N // 2            # 16 entity rows per core
P = NI * N             # 512 pairs per core
PC = P // 128          # 4 pair chunks
NS = S // 128          # 8 s tiles
HC = H // 128          # 6 hidden chunks
ACH = AD // 128        # 2
PQ = DD * DD           # 144

f32 = mybir.dt.float32
bf16 = mybir.dt.bfloat16
f32r = mybir.dt.float32r

_CACHE = {}


def _r(ap):
    return ap.bitcast(f32r)


def build_program(coll=True):
    nc = bacc.Bacc("TRN2", target_bir_lowering=False, debug=False, num_devices=8)

    # ---- external I/O ----
    ga_all_d = nc.dram_tensor("ga_all", [S, N * HH], bf16, kind="ExternalInput").ap()
    ga_mine_d = nc.dram_tensor("ga_mine", [S, NI * HH], bf16, kind="ExternalInput").ap()
    seqT_d = nc.dram_tensor("seqT", [H, S], bf16, kind="ExternalInput").ap()
    sgT_d = nc.dram_tensor("sgT", [H, NI * M], f32, kind="ExternalInput").ap()
    Ws_d = nc.dram_tensor("Ws", [H, Wd], bf16, kind="ExternalInput").ap()
    Wo_d = nc.dram_tensor("Wo", [H, Wd], bf16, kind="ExternalInput").ap()
    Wc_d = nc.dram_tensor("Wc", [H, Wd], bf16, kind="ExternalInput").ap()
    Af_d = nc.dram_tensor("Af", [Wd, PQ], bf16, kind="ExternalInput").ap()
    bbl_d = nc.dram_tensor("bbl", [Wd], f32, kind="ExternalInput").ap()
    Wat_d = nc.dram_tensor("Wat", [Wd, AD], bf16, kind="ExternalInput").ap()
    vat_d = nc.dram_tensor("vat", [AD], bf16, kind="ExternalInput").ap()
    clfW_d = nc.dram_tensor("clfW", [Wd, C], bf16, kind="ExternalInput").ap()
    clfb_d = nc.dram_tensor("clfb", [C], f32, kind="ExternalInput").ap()
    eye128_d = nc.dram_tensor("eye128", [128, 128], f32, kind="ExternalInput").ap()
    eyerow_d = nc.dram_tensor("eyerow", [P, NI], bf16, kind="ExternalInput").ap()
    eyecol_d = nc.dram_tensor("eyecol", [P, N], bf16, kind="ExternalInput").ap()
    eyerowT_d = nc.dram_tensor("eyerowT", [NI, P], bf16, kind="ExternalInput").ap()
    eyecolT_d = nc.dram_tensor("eyecolT", [N, P], bf16, kind="ExternalInput").ap()
    out_d = nc.dram_tensor("out", [P, C], f32, kind="ExternalOutput").ap()

    # ---- internal DRAM (staging + collectives) ----
    rs_dd = nc.dram_tensor("rs_dd", [NI, Wd], bf16).ap()
    ro_dd = nc.dram_tensor("ro_dd", [NI, Wd], bf16).ap()
    cw_dd = nc.dram_tensor("cw_dd", [AD], f32).ap()
    # cols 0:C = unnormalized classified column contexts, col C = exp colsums
    cc_dd = nc.dram_tensor("cc_dd", [N, C + 1], f32).ap()
    ccr_dd = nc.dram_tensor("ccr_dd", [N, C + 1], f32).ap()

    GROUPS = [[0, 1], [2, 3], [4, 5], [6, 7]]

    with tile.TileContext(nc) as tc:
        with ExitStack() as top:
            top.enter_context(nc.allow_low_precision(
                "bf16/f32r staging throughout; all contractions accumulate fp32 in PSUM"))
            consts = top.enter_context(tc.tile_pool(name="consts", bufs=1))
            late = top.enter_context(tc.tile_pool(name="late", bufs=1))
            Tp = top.enter_context(tc.tile_pool(name="Tp", bufs=1))

            T_sb = late.tile([128, PC, PQ], f32, tag="T")
            recip_pp = late.tile([128, PC], f32, tag="recip_pp")
            exp_pp = late.tile([128, PC], f32, tag="exp_pp")
            exp_bf = late.tile([128, PC], bf16, tag="exp_bf")
            zs_sb = late.tile([128, PC, Wd], bf16, tag="zs_z")
            zo_sb = late.tile([128, PC, Wd], bf16, tag="zo_z")
            AW1_sb = late.tile([128, AD], bf16, tag="AW1")
            AW2_sb = late.tile([16, AD], bf16, tag="AW2")
            ACl1_sb = late.tile([128, C], bf16, tag="ACl1")
            ACl2_sb = late.tile([16, C], bf16, tag="ACl2")
            cw_pp = late.tile([128, ACH], f32, tag="cw_pp")
            biasFrow = late.tile([1, C], bf16, tag="biasFrow")

            with tc.tile_pool(name="Ep", bufs=1) as Ep, \
                 tc.tile_pool(name="zstg", bufs=2) as zstg, \
                 tc.tile_pool(name="midp", bufs=1) as midp:

                # ============ input DMAs, ordered by consumer urgency ========
                ga_sb = midp.tile([128, NS, N * HH], bf16, tag="ga")
                gam_sb = midp.tile([128, NS, NI * HH], bf16, tag="gam")
                # first s-tile of ga/gam lands first so pa can start early
                nc.sync.dma_start(ga_sb[:, 0, :], ga_all_d[0:128])
                nc.scalar.dma_start(gam_sb[:, 0, :], ga_mine_d[0:128])
                nc.sync.dma_start(
                    ga_sb[:, 1:NS, :],
                    ga_all_d[128:S].rearrange("(t p) f -> p t f", p=128))
                nc.scalar.dma_start(
                    gam_sb[:, 1:NS, :],
                    ga_mine_d[128:S].rearrange("(t p) f -> p t f", p=128))

                seqT_sb = midp.tile([128, HC, S], bf16, tag="seqT")
                nc.sync.dma_start(seqT_sb[:], seqT_d.rearrange("(c p) s -> p c s", p=128))
                Wc_sb = consts.tile([128, HC, Wd], bf16)
                nc.sync.dma_start(Wc_sb[:], Wc_d.rearrange("(c p) w -> p c w", p=128))
                sgT_sb = midp.tile([128, HC, NI * M], f32, tag="sgT")
                nc.sync.dma_start(sgT_sb[:], sgT_d.rearrange("(c p) f -> p c f", p=128))
                Wso_sb = midp.tile([128, 2, HC, Wd], bf16, tag="Wso")
                nc.sync.dma_start(Wso_sb[:, 0], Ws_d.rearrange("(c p) w -> p c w", p=128))
                nc.sync.dma_start(Wso_sb[:, 1], Wo_d.rearrange("(c p) w -> p c w", p=128))

                Af_sb = midp.tile([128, HC, PQ], bf16, tag="Af")
                nc.sync.dma_start(Af_sb[:], Af_d.rearrange("(c p) q -> p c q", p=128))
                Wat_sb = midp.tile([128, HC, AD], bf16, tag="Wat")
                nc.sync.dma_start(Wat_sb[:], Wat_d.rearrange("(c p) a -> p c a", p=128))
                clf_sb = midp.tile([128, HC, C], bf16, tag="clf")
                nc.sync.dma_start(clf_sb[:], clfW_d.rearrange("(c p) o -> p c o", p=128))
                eye_sb = consts.tile([128, 128], f32)
                nc.sync.dma_start(eye_sb[:], eye128_d[:])
                eyerow_sb = consts.tile([128, PC, NI], bf16)
                nc.sync.dma_start(eyerow_sb[:], eyerow_d.rearrange("(c p) i -> p c i", p=128))
                eyecol_sb = consts.tile([128, PC, N], bf16)
                nc.sync.dma_start(eyecol_sb[:], eyecol_d.rearrange("(c p) j -> p c j", p=128))
                eyerowT_sb = consts.tile([NI, PC, 128], bf16)
                nc.sync.dma_start(eyerowT_sb[:], eyerowT_d.rearrange("i (c p) -> i c p", p=128))
                eyecolT_sb = consts.tile([N, PC, 128], bf16)
                nc.sync.dma_start(eyecolT_sb[:], eyecolT_d.rearrange("j (c p) -> j c p", p=128))
                b_sb = midp.tile([128, HC], f32, tag="b_sb")
                nc.sync.dma_start(b_sb[:], bbl_d.rearrange("(c p) -> p c", p=128))
                v_sb = consts.tile([128, ACH], bf16)
                nc.sync.dma_start(v_sb[:], vat_d.rearrange("(c p) -> p c", p=128))
                clfb_row = midp.tile([1, C], f32, tag="clfb_row")
                nc.sync.dma_start(clfb_row[:], clfb_d.unsqueeze(0))
                ones_row = consts.tile([1, 128], bf16)
                nc.vector.memset(ones_row[:], 1.0)


                # SWc rhs gets a ones column at index Wd -> ctx matmul also
                # produces the pa row-sums per pair partition.
                SWc_sb = midp.tile([128, NS, Wd + 1], bf16, tag="SWc")
                nc.vector.memset(SWc_sb[:, :, Wd:Wd + 1], 1.0)

                paT_sb = midp.tile([128, NS, P], bf16, tag="paT")

                with tc.tile_pool(name="psmain", bufs=4, space="PSUM") as psmain:
                    # rep logsumexp front (Act; its Pool reduce is emitted
                    # inside the pa loop so it slots between the Pool tails)
                    expg = midp.tile([128, HC * NI * M], f32, tag="expg")
                    nc.scalar.activation(expg[:], sgT_sb[:].rearrange("p c f -> p (c f)"),
                                         mybir.ActivationFunctionType.Exp)
                    rsum_m = midp.tile([128, HC * NI], f32, tag="rsum_m")
                    repT_sb = midp.tile([128, HC, NI], bf16, tag="repT")
                    egv = expg[:].rearrange("p (g m) -> p g m", m=M)
                    rs2 = midp.tile([128, HC * NI * 2], f32, tag="rs2")
                    rs2v = rs2[:].rearrange("p (g m) -> p g m", m=2)
                    nc.gpsimd.tensor_add(rs2v, egv[:, :, 0:2], egv[:, :, 2:4])
                    nc.gpsimd.tensor_add(rsum_m[:], rs2v[:, :, 0].squeeze(),
                                         rs2v[:, :, 1].squeeze())
                    with nc.allow_low_precision("bf16 rep"):
                        nc.scalar.activation(
                            repT_sb[:].rearrange("p c i -> p (c i)"), rsum_m[:],
                            mybir.ActivationFunctionType.Ln)

                    # ---- SW_c = seq @ W_c ----
                    for t in range(NS):
                        sw_ps = psmain.tile([128, Wd + 1], f32, tag="ps")
                        for hc in range(HC):
                            lhsT = seqT_sb[:, hc, t * 128:(t + 1) * 128]
                            nc.tensor.matmul(sw_ps[:, 0:512], lhsT, Wc_sb[:, hc, 0:512],
                                             start=(hc == 0), stop=(hc == HC - 1))
                            nc.tensor.matmul(sw_ps[:, 512:768], lhsT, Wc_sb[:, hc, 512:768],
                                             start=(hc == 0), stop=(hc == HC - 1))
                        with nc.allow_low_precision("bf16 SWc"):
                            nc.scalar.copy(SWc_sb[:, t, 0:Wd], sw_ps[:, 0:Wd])

                    # ---- rep @ Ws / Wo ----
                    for wi, r_dd in ((0, rs_dd), (1, ro_dd)):
                        r_ps = psmain.tile([16, Wd + 1], f32, tag="ps")
                        for k in range(HC):
                            nc.tensor.matmul(r_ps[:, 0:512], repT_sb[:, k, :],
                                             Wso_sb[:, wi, k, 0:512], start=(k == 0), stop=(k == HC - 1))
                            nc.tensor.matmul(r_ps[:, 512:768], repT_sb[:, k, :],
                                             Wso_sb[:, wi, k, 512:768], start=(k == 0), stop=(k == HC - 1))
                        r_row = midp.tile([16, Wd], bf16, tag="r_row", bufs=2)
                        with nc.allow_low_precision("bf16 rs/ro staging"):
                            nc.scalar.copy(r_row[:], r_ps[:, 0:Wd])
                        nc.sync.dma_start(r_dd[:], r_row[:])

                    # preload the Tanh act table during the idle window
                    dummy_th = midp.tile([1, 1], f32, tag="dummy_th")
                    nc.scalar.activation(dummy_th[:], r_row[0:1, 0:1],
                                         mybir.ActivationFunctionType.Tanh)

                    # ---- weight pre-contractions (PE shadow work) ----
                    # AW = Af @ W_attn [PQ, AD]; ACl = Af @ clf_W [PQ, C];
                    # cw = (K b) @ W_attn; bf = (3 K b) @ clf_W
                    bb_sb = midp.tile([128, HC], bf16, tag="bb_sb")
                    bb3_sb = midp.tile([128, HC], bf16, tag="bb3_sb")
                    with nc.allow_low_precision("bf16 bias staging"):
                        nc.scalar.mul(bb_sb[:], b_sb[:], float(KK))
                        nc.scalar.mul(bb3_sb[:], b_sb[:], 3.0 * KK)
                    # fp32r matmul outputs must start on a PSUM bank
                    # boundary, so the clf-side products live at column 512
                    for lo, hi, aw_sb, acl_sb in ((0, 128, AW1_sb, ACl1_sb),
                                                  (128, 144, AW2_sb, ACl2_sb)):
                        aw_ps = psmain.tile([hi - lo, Wd + 1], f32, tag="ps")
                        for wc in range(HC):
                            nc.tensor.matmul(aw_ps[:, 0:AD], Af_sb[:, wc, lo:hi],
                                             Wat_sb[:, wc, :],
                                             start=(wc == 0), stop=(wc == HC - 1))
                            nc.tensor.matmul(aw_ps[:, 512:512 + C], Af_sb[:, wc, lo:hi],
                                             clf_sb[:, wc, :],
                                             start=(wc == 0), stop=(wc == HC - 1))
                        with nc.allow_low_precision("bf16 AW/ACl"):
                            nc.scalar.copy(aw_sb[:], aw_ps[:, 0:AD])
                            nc.scalar.copy(acl_sb[:], aw_ps[:, 512:512 + C])
                    cwb_ps = psmain.tile([1, Wd + 1], f32, tag="ps")
                    for hc in range(HC):
                        nc.tensor.matmul(cwb_ps[:, 0:AD], bb_sb[:, hc:hc + 1],
                                         Wat_sb[:, hc, :],
                                         start=(hc == 0), stop=(hc == HC - 1))
                        nc.tensor.matmul(cwb_ps[:, 512:512 + C], bb3_sb[:, hc:hc + 1],
                                         clf_sb[:, hc, :],
                                         start=(hc == 0), stop=(hc == HC - 1))
                    cw_row = midp.tile([1, AD], f32, tag="cw_row")
                    nc.scalar.copy(cw_row[:], cwb_ps[:, 0:AD])
                    nc.sync.dma_start(cw_dd.unsqueeze(0), cw_row[:])
                    nc.sync.dma_start(cw_pp[:], cw_dd.rearrange("(c p) -> p c", c=ACH))
                    with nc.allow_low_precision("bf16 bias"):
                        nc.vector.scalar_tensor_tensor(
                            biasFrow[:], cwb_ps[:, 512:512 + C], 1.0, clfb_row[:],
                            op0=mybir.AluOpType.mult, op1=mybir.AluOpType.add)

                    # rs/ro replicated across the pair partitions
                    rrep = {}
                    for nm, r_dd in (("rs", rs_dd), ("ro", ro_dd)):
                        rr = midp.tile([128, PC, Wd], bf16, tag=f"{nm}_rep")
                        for pc in range(PC):
                            nc.sync.dma_start(
                                rr[:, pc, :],
                                r_dd[pc * 4:(pc + 1) * 4].unsqueeze(1).broadcast_to((4, N, Wd)))
                        rrep[nm] = rr

                    # ---- pa products (DVE + Pool tail) + ctx matmuls chasing ----
                    ctxps = [psmain.tile([128, Wd + 1], f32, tag="ps", name=f"ctx{pc}")
                             for pc in range(PC)]
                    for t in range(NS):
                        E = Ep.tile([128, NI * N * HH], bf16, tag="E")
                        nc.vector.tensor_mul(
                            E[:].rearrange("p (i j h) -> p i j h", i=NI, j=N),
                            gam_sb[:, t, :].rearrange("p (i h) -> p i h", i=NI)
                                .unsqueeze(2).broadcast_to((128, NI, N, HH)),
                            ga_sb[:, t, :].rearrange("p (j h) -> p j h", j=N)
                                .unsqueeze(1).broadcast_to((128, NI, N, HH)))
                        # pairwise tree sum over h=12 (2x bf16 on DVE; Pool
                        # takes the tails, plus r2 on some tiles for balance)
                        Ev = E[:].rearrange("p (ij h) -> p ij h", h=HH)
                        r1 = Ep.tile([128, P * 6], bf16, tag="ttA")
                        r1v = r1[:].rearrange("p (ij h) -> p ij h", h=6)
                        nc.vector.tensor_add(r1v, Ev[:, :, 0:6], Ev[:, :, 6:12])
                        r2 = Ep.tile([128, P * 3], bf16, tag="ttB", bufs=2)
                        r2v = r2[:].rearrange("p (ij h) -> p ij h", h=3)
                        r2eng = nc.gpsimd if t in (0, 2, 4, 6) else nc.vector
                        r2eng.tensor_add(r2v, r1v[:, :, 0:3], r1v[:, :, 3:6])
                        teng = nc.vector if t == NS - 1 else nc.gpsimd
                        r3 = Ep.tile([128, P], bf16, tag="r3", bufs=2)
                        teng.tensor_add(r3[:], r2v[:, :, 0].squeeze(), r2v[:, :, 1].squeeze())
                        with nc.allow_low_precision("bf16 pa tree"):
                            teng.tensor_add(paT_sb[:, t, :], r3[:], r2v[:, :, 2].squeeze())
                        for pc in range(PC):
                            lhsT = paT_sb[:, t, pc * 128:(pc + 1) * 128]
                            nc.tensor.matmul(ctxps[pc][:, 0:512], lhsT, SWc_sb[:, t, 0:512],
                                             start=(t == 0), stop=(t == NS - 1))
                            nc.tensor.matmul(ctxps[pc][:, 512:Wd + 1], lhsT, SWc_sb[:, t, 512:Wd + 1],
                                             start=(t == 0), stop=(t == NS - 1))

                    # ---- z = tanh((ctx + rsum * repW) / rsum), interleaved with
                    # the bilinear T products so DVE never idles on Act ----
                    for pc in range(PC):
                        nc.vector.reciprocal(recip_pp[:, pc:pc + 1], ctxps[pc][:, Wd:Wd + 1])

                    def emit_z(pc):
                        # pc0: tanh((ctx + rsum*repW)/rsum) via two DVE STTs
                        # (shortest chain; DVE idles right after pa).
                        # pc>0: Act pre-scales ctx by 1/rsum to bf16, DVE does
                        # only a 2x-mode bf16 add, Act applies tanh.
                        if pc == 0:
                            for z_sb, rrk in ((zs_sb, "rs"), (zo_sb, "ro")):
                                zpre = zstg.tile([128, Wd], f32, tag="zpre")
                                nc.vector.scalar_tensor_tensor(
                                    zpre[:], rrep[rrk][:, pc, :], ctxps[pc][:, Wd:Wd + 1],
                                    ctxps[pc][:, 0:Wd],
                                    op0=mybir.AluOpType.mult, op1=mybir.AluOpType.add)
                                with nc.allow_low_precision("bf16 z"):
                                    nc.scalar.activation(z_sb[:, pc, :], zpre[:],
                                                         mybir.ActivationFunctionType.Tanh,
                                                         scale=recip_pp[:, pc:pc + 1])
                            return
                        zsc = zstg.tile([128, Wd], bf16, tag="zsc")
                        with nc.allow_low_precision("bf16 ctx scale"):
                            nc.scalar.mul(zsc[:], ctxps[pc][:, 0:Wd], recip_pp[:, pc:pc + 1])
                        for z_sb, rrk in ((zs_sb, "rs"), (zo_sb, "ro")):
                            zpre = zstg.tile([128, Wd], bf16, tag="zpre2")
                            with nc.allow_low_precision("bf16 z"):
                                nc.vector.tensor_add(zpre[:], zsc[:], rrep[rrk][:, pc, :])
                            nc.scalar.activation(z_sb[:, pc, :], zpre[:],
                                                 mybir.ActivationFunctionType.Tanh)

                    def emit_tprod(pc):
                        ET = Tp.tile([128, PQ * KK], bf16, tag="E2")
                        nc.vector.tensor_mul(
                            ET[:].rearrange("p (a q k) -> p a q k", a=DD, q=DD),
                            zs_sb[:, pc, :].rearrange("p (a k) -> p a k", a=DD)
                                .unsqueeze(2).broadcast_to((128, DD, DD, KK)),
                            zo_sb[:, pc, :].rearrange("p (q k) -> p q k", q=DD)
                                .unsqueeze(1).broadcast_to((128, DD, DD, KK)))
                        cur = ET[:].rearrange("p (pq k) -> p pq k", k=KK)
                        w = KK
                        ab = 0
                        # last chunk: keep the tail on DVE (it idles after the
                        # multiplies; queued-up Pool would sit on the critical path)
                        tail_eng = nc.vector if pc == PC - 1 else nc.gpsimd
                        while w > 2:
                            w //= 2
                            eng = nc.vector if w >= 16 else tail_eng
                            nxt = Tp.tile([128, PQ * w], bf16, tag=f"tt{'AB'[ab % 2]}")
                            ab += 1
                            nxtv = nxt[:].rearrange("p (pq k) -> p pq k", k=w)
                            eng.tensor_add(nxtv, cur[:, :, 0:w], cur[:, :, w:2 * w])
                            cur = nxtv
                        with nc.allow_low_precision("bf16 T tree"):
                            tail_eng.tensor_add(T_sb[:, pc, :], cur[:, :, 0].squeeze(),
                                                cur[:, :, 1].squeeze())

                    emit_z(0)
                    emit_z(1)
                    emit_tprod(0)
                    emit_z(2)
                    emit_tprod(1)
                    emit_z(3)
                    emit_tprod(2)
                    emit_tprod(3)

            # ============ post-T chase: everything bf16, pair-major ============
            with tc.tile_pool(name="pstp", bufs=1, space="PSUM") as pstp, \
                 tc.tile_pool(name="psmm", bufs=3, space="PSUM") as psmm, \
                 tc.tile_pool(name="psacc", bufs=1, space="PSUM") as psacc:

                # one accumulation group per PSUM bank (start zeroes the
                # whole 2KB region): row/col context contractions and the exp
                # row/col sums each get their own bank with a plain
                # start-at-pc0 / stop-at-pc3 group
                mtr = psacc.tile([NI, C], f32, tag="mtr")
                mtc = psacc.tile([N, C], f32, tag="mtc")
                rsum_ps = psacc.tile([NI, 1], f32, tag="rsum")
                csum_ps = psacc.tile([N, 1], f32, tag="csum")

                TT1_sb = late.tile([128, P], bf16, tag="TT1")
                TT2_sb = late.tile([16, P], bf16, tag="TT2")
                gclf_sb = late.tile([128, PC, C], bf16, tag="gclf")
                th_sb = late.tile([128, ACH, P], bf16, tag="th")
                Mrc_sb = late.tile([128, PC, NI + N], bf16, tag="Mrc")

                def emit_tp(pc):
                    tp1 = pstp.tile([128, 128], f32, tag="tp")
                    nc.tensor.transpose(tp1[:], T_sb[:, pc, 0:128], eye_sb[:])
                    with nc.allow_low_precision("bf16 TT"):
                        nc.scalar.copy(TT1_sb[:, pc * 128:(pc + 1) * 128], tp1[:])
                    tp2 = pstp.tile([16, 128], f32, tag="tp")
                    nc.tensor.transpose(tp2[:], T_sb[:, pc, 128:144], eye_sb[:])
                    with nc.allow_low_precision("bf16 TT"):
                        nc.scalar.copy(TT2_sb[:, pc * 128:(pc + 1) * 128], tp2[:])

                def emit_gclf(pc):
                    # per-pair classifier logits of g (sans contexts/bias),
                    # both into the output accumulator and as a bf16 copy for
                    # the row/col context contraction
                    sl = slice(pc * 128, (pc + 1) * 128)
                    gp = psmm.tile([128, 512], f32, tag="mm")
                    nc.tensor.matmul(gp[:, 0:C], TT1_sb[:, sl], ACl1_sb[:],
                                     start=True, stop=False)
                    nc.tensor.matmul(gp[:, 0:C], TT2_sb[:, sl], ACl2_sb[:],
                                     start=False, stop=True)
                    with nc.allow_low_precision("bf16 gclf"):
                        nc.scalar.copy(gclf_sb[:, pc, :], gp[:, 0:C])

                def emit_th_e(pcs):
                    # th = tanh(T @ AW + cw), per pair chunk (128-wide bf16
                    # matmuls run at full rate), then e + exp per chunk
                    for pc in pcs:
                        sl = slice(pc * 128, (pc + 1) * 128)
                        for ac in range(ACH):
                            ep_ps = psmm.tile([128, 512], f32, tag="mm")
                            nc.tensor.matmul(ep_ps[:, 0:128], AW1_sb[:, ac * 128:(ac + 1) * 128],
                                             TT1_sb[:, sl], start=True, stop=False)
                            nc.tensor.matmul(ep_ps[:, 0:128], AW2_sb[:, ac * 128:(ac + 1) * 128],
                                             TT2_sb[:, sl], start=False, stop=True)
                            with nc.allow_low_precision("bf16 th"):
                                nc.scalar.activation(th_sb[:, ac, sl], ep_ps[:, 0:128],
                                                     mybir.ActivationFunctionType.Tanh,
                                                     bias=cw_pp[:, ac:ac + 1])
                        e_ps = psmm.tile([128, 512], f32, tag="mm")
                        for ac in range(ACH):
                            nc.tensor.matmul(e_ps[:, 0:1],
                                             th_sb[:, ac, sl],
                                             v_sb[:, ac:ac + 1],
                                             start=(ac == 0), stop=(ac == ACH - 1))
                        nc.scalar.activation(exp_pp[:, pc:pc + 1], e_ps[:, 0:1],
                                             mybir.ActivationFunctionType.Exp)
                        with nc.allow_low_precision("bf16 exp"):
                            nc.gpsimd.tensor_copy(exp_bf[:, pc:pc + 1], exp_pp[:, pc:pc + 1])

                def emit_sel(pcs):
                    """selectors, exp row/col sums, Mrc^T gclf accumulation.
                    The last pair's selector muls run on DVE (idle by then);
                    the first pair's on GPSIMD (DVE still on tree products)."""
                    for pc in pcs:
                        # per-partition scale-by-exp runs on the scalar engine
                        with nc.allow_low_precision("bf16 selectors"):
                            nc.scalar.mul(Mrc_sb[:, pc, 0:NI], eyerow_sb[:, pc, :],
                                          exp_pp[:, pc:pc + 1])
                            nc.scalar.mul(Mrc_sb[:, pc, NI:NI + N], eyecol_sb[:, pc, :],
                                          exp_pp[:, pc:pc + 1])
                        nc.tensor.matmul(rsum_ps[:], eyerow_sb[:, pc, :],
                                         exp_bf[:, pc:pc + 1],
                                         start=(pc == 0), stop=(pc == PC - 1))
                        nc.tensor.matmul(csum_ps[:], eyecol_sb[:, pc, :],
                                         exp_bf[:, pc:pc + 1],
                                         start=(pc == 0), stop=(pc == PC - 1))
                        nc.tensor.matmul(mtr[:], Mrc_sb[:, pc, 0:NI],
                                         gclf_sb[:, pc, :],
                                         start=(pc == 0), stop=(pc == PC - 1))
                        nc.tensor.matmul(mtc[:], Mrc_sb[:, pc, NI:NI + N],
                                         gclf_sb[:, pc, :],
                                         start=(pc == 0), stop=(pc == PC - 1))

                emit_tp(0)
                emit_tp(1)
                emit_gclf(0)
                emit_gclf(1)
                emit_th_e((0, 1))
                emit_sel((0, 1))
                emit_tp(2)
                emit_tp(3)
                emit_gclf(2)
                emit_gclf(3)
                emit_th_e((2, 3))
                emit_sel((2, 3))

                # ---- local sums out; one combined pair AllReduce ----
                cs_col = late.tile([N, 1], f32, tag="cs_col")
                nc.scalar.copy(cs_col[:], csum_ps[:])
                nc.sync.dma_start(cc_dd[0:N, C:C + 1], cs_col[:])
                ccu_sb = late.tile([N, C], f32, tag="ccu")
                nc.scalar.copy(ccu_sb[:], mtc[:])
                nc.sync.dma_start(cc_dd[0:N, 0:C], ccu_sb[:])
                if coll:
                    nc.gpsimd.collective_compute("AllReduce", mybir.AluOpType.add,
                                                 replica_groups=GROUPS, ins=[cc_dd[:]],
                                                 outs=[ccr_dd[:]])
                else:
                    nc.sync.dma_start(ccr_dd[0:N, C:C + 1], cs_col[:])
                    nc.sync.dma_start(ccr_dd[0:N, 0:C], ccu_sb[:])
                # ---- local row path (overlaps the collective) ----
                rrec_col = late.tile([NI, 1], f32, tag="rrec_col")
                nc.vector.reciprocal(rrec_col[:], rsum_ps[:])
                rcn_r = late.tile([NI, C], bf16, tag="rcn_r")
                with nc.allow_low_precision("bf16 contexts"):
                    nc.vector.tensor_scalar_mul(rcn_r[:], mtr[:], rrec_col[:])

                # ---- column path after the collective lands ----
                csg_col = late.tile([N, 1], f32, tag="csg_col")
                nc.sync.dma_start(csg_col[:], ccr_dd[0:N, C:C + 1])
                ccg_sb = late.tile([N, C], f32, tag="ccg")
                nc.scalar.dma_start(ccg_sb[:], ccr_dd[0:N, 0:C])
                crec_col = late.tile([N, 1], f32, tag="crec_col")
                nc.vector.reciprocal(crec_col[:], csg_col[:])
                rcn_c = late.tile([N, C], bf16, tag="rcn_c")
                with nc.allow_low_precision("bf16 contexts"):
                    nc.vector.tensor_scalar_mul(rcn_c[:], ccg_sb[:], crec_col[:])
                out_sb = late.tile([128, PC, C], f32, tag="out_sb")
                for pc in range(PC):
                    cx = psmm.tile([128, 512], f32, tag="mm")
                    nc.tensor.matmul(cx[:, 0:C], eyerowT_sb[:, pc, :], rcn_r[:],
                                     start=True, stop=False)
                    nc.tensor.matmul(cx[:, 0:C], ones_row[:], biasFrow[:],
                                     start=False, stop=False)
                    nc.tensor.matmul(cx[:, 0:C], eyecolT_sb[:, pc, :], rcn_c[:],
                                     start=False, stop=True)
                    nc.vector.tensor_add(out_sb[:, pc, :], cx[:, 0:C], gclf_sb[:, pc, :])
                nc.sync.dma_start(out_d.rearrange("(c p) o -> p c o", p=128),
                                  out_sb[:])

    nc.compile()
    return nc


def host_shard(inputs):
    seq_all = np.ascontiguousarray(inputs["sequence_output"], dtype=np.float32)
    att_all = np.ascontiguousarray(inputs["attention"], dtype=np.float32)
    ep_all = np.asarray(inputs["entity_pos"])
    W_s = np.ascontiguousarray(inputs["W_s"], dtype=np.float32)
    W_o = np.ascontiguousarray(inputs["W_o"], dtype=np.float32)
    W_c = np.ascontiguousarray(inputs["W_c"], dtype=np.float32)
    A_bl = np.ascontiguousarray(inputs["A_bl"], dtype=np.float32)
    b_bl = np.ascontiguousarray(inputs["b_bl"], dtype=np.float32)
    W_attn = np.ascontiguousarray(inputs["W_attn"], dtype=np.float32)
    v_attn = np.ascontiguousarray(inputs["v_attn"], dtype=np.float32)
    clf_W = np.ascontiguousarray(inputs["clf_W"], dtype=np.float32)
    clf_b = np.ascontiguousarray(inputs["clf_b"], dtype=np.float32)

    Af = np.ascontiguousarray(A_bl.reshape(Wd, PQ))               # [w, (p,q)]
    eye128 = np.eye(128, dtype=np.float32)
    eyerow = np.repeat(np.eye(NI, dtype=np.float32), N, axis=0)   # [P, NI]
    eyecol = np.tile(np.eye(N, dtype=np.float32), (NI, 1))        # [P, N]

    shared = dict(Ws=W_s.astype(ml_dtypes.bfloat16), Wo=W_o.astype(ml_dtypes.bfloat16),
                  Wc=W_c.astype(ml_dtypes.bfloat16), Af=Af.astype(ml_dtypes.bfloat16),
                  bbl=b_bl, Wat=W_attn.astype(ml_dtypes.bfloat16),
                  vat=v_attn.astype(ml_dtypes.bfloat16),
                  clfW=clf_W.astype(ml_dtypes.bfloat16), clfb=clf_b,
                  eye128=eye128, eyerow=eyerow.astype(ml_dtypes.bfloat16),
                  eyecol=eyecol.astype(ml_dtypes.bfloat16),
                  eyerowT=np.ascontiguousarray(eyerow.T).astype(ml_dtypes.bfloat16),
                  eyecolT=np.ascontiguousarray(eyecol.T).astype(ml_dtypes.bfloat16))

    in_maps = []
    for doc in range(B):
        seq = seq_all[doc]
        ep = ep_all[doc].astype(np.int64)
        ga = att_all[doc][:, ep.reshape(-1), :].reshape(HH, N, M, S).sum(axis=2)  # [h, n, s]
        ga_T = np.ascontiguousarray(ga.transpose(2, 1, 0).reshape(S, N * HH)).astype(
            ml_dtypes.bfloat16)                                                    # [s, (n,h)]
        seq_gT = np.ascontiguousarray(seq[ep.reshape(-1), :].T)                    # [H, N*M]
        for half in range(2):
            i0 = half * NI
            m = dict(shared)
            m["ga_all"] = ga_T
            m["ga_mine"] = np.ascontiguousarray(ga_T[:, i0 * HH:(i0 + NI) * HH])
            m["seqT"] = np.ascontiguousarray(seq.T).astype(ml_dtypes.bfloat16)
            m["sgT"] = np.ascontiguousarray(seq_gT[:, i0 * M:(i0 + NI) * M])
            in_maps.append(m)
    return in_maps


def assemble(results):
    out = np.empty((B, N, N, C), np.float32)
    for core in range(2 * B):
        doc, half = divmod(core, 2)
        out[doc, half * NI:(half + 1) * NI] = results[core]["out"].reshape(NI, N, C)
    return out


def kernel(**inputs):
    nc = _CACHE.get("nc")
    if nc is None:
        nc = build_program()
        _CACHE["nc"] = nc
    in_maps = host_shard(inputs)
    res = run_bass_kernel_spmd(nc, in_maps, list(range(2 * B)))
    _CACHE["last_res"] = res
    return assemble(res.results)



# revision 2
# speedup vs baseline: 2.6718x; 2.6718x over previous
"""DocRE (ATLOP-style) Trainium2 kernel, v4.

Sharding: data-parallel over the 4 documents x 2-way split of the entity-row
axis (i) -> 8 cores.  Each core computes the full pipeline for its 16x32
entity-pair block; cross-core coupling (column softmax + column context) is
handled with a small pair AllReduce; softmax normalizations fold in after.

v4 changes over v3:
- ga_mine input dropped: per-core ga column rotation on host puts the core's
  own 16 entities first, so the i-side of the pa product is a slice of ga.
  (eyecol/eyecolT carry the inverse rotation so the collective stays aligned
  on global j; assemble() un-rotates.)
- bilinear tree rebalanced: chunks 0-2 put only the k=32 level on DVE and the
  rest on GPSIMD; the last chunk runs entirely on DVE so the final T write is
  never gated on the slower GPSIMD tail.
- tail: single combined DMA for the returning collective payload, split
  output DMA so the first pair-chunks fly while the last is still summing.
- build_program(reps=N) emits the body N times (internal DRAM staging per
  rep) for slope-timing the true per-exec device time.
"""

import os
import sys
from contextlib import ExitStack

import numpy as np

for _p in ("/opt/trn_rl_repo", "/root/.axon_site/_ro/trn_rl_repo"):
    if os.path.isdir(_p) and _p not in sys.path:
        sys.path.insert(0, _p)

import ml_dtypes  # noqa: E402
import concourse.bass as bass  # noqa: E402,F401
import concourse.tile as tile  # noqa: E402
from concourse import bacc, mybir  # noqa: E402
from concourse.bass_utils import run_bass_kernel_spmd  # noqa: E402

# problem dims (hardcoded per spec)
B, S, H, HH, N, M = 4, 1024, 768, 12, 32, 4
Wd, AD, C, KK = 768, 256, 97, 64
DD = Wd // KK          # 12
NI = N // 2            # 16 entity rows per core
P = NI * N             # 512 pairs per core
PC = P // 128          # 4 pair chunks
NS = S // 128          # 8 s tiles
HC = H // 128          # 6 hidden chunks
ACH = AD // 128        # 2
PQ = DD * DD           # 144

f32 = mybir.dt.float32
bf16 = mybir.dt.bfloat16

_CACHE = {}

GROUPS = [[0, 1], [2, 3], [4, 5], [6, 7]]


# pieces packed into the two input blobs: (key, elems). Offsets are the
# running sums; host_shard packs in exactly this order.
BF_PIECES = [
    ("ga_all", S * N * HH), ("seqT", H * S), ("Ws", H * Wd), ("Wo", H * Wd),
    ("Wc", H * Wd), ("Af", Wd * PQ), ("Wat", Wd * AD), ("vat", AD),
    ("clfW", Wd * C), ("eyerow", P * NI), ("eyecol", P * N),
    ("eyerowT", NI * P), ("eyecolT", N * P),
]
F32_PIECES = [("sgT", H * NI * M), ("bbl", Wd), ("clfb", C), ("eye128", 128 * 128)]


def _offsets(pieces):
    off, d = 0, {}
    for k, n in pieces:
        d[k] = (off, off + n)
        off += n
    return d, off


BF_OFF, BF_LEN = _offsets(BF_PIECES)
F32_OFF, F32_LEN = _offsets(F32_PIECES)


def _make_io(nc):
    # single bf16 + single f32 input blob: per-call host dispatch overhead
    # scales with the NUMBER of input buffers (~30us/buffer/exec through the
    # PJRT tunnel), so everything rides in two tensors.
    d = {}
    d["blob_bf"] = nc.dram_tensor("blob_bf", [BF_LEN], bf16, kind="ExternalInput").ap()
    d["blob_f32"] = nc.dram_tensor("blob_f32", [F32_LEN], f32, kind="ExternalInput").ap()
    d["out"] = nc.dram_tensor("out", [P, C], f32, kind="ExternalOutput").ap()
    return d


def _make_internal(nc, r):
    d = {}
    d["rs_dd"] = nc.dram_tensor(f"rs_dd{r}", [NI, Wd], bf16).ap()
    d["ro_dd"] = nc.dram_tensor(f"ro_dd{r}", [NI, Wd], bf16).ap()
    d["cw_dd"] = nc.dram_tensor(f"cw_dd{r}", [AD], f32).ap()
    # cols 0:C = unnormalized classified column contexts, col C = exp colsums
    d["cc_dd"] = nc.dram_tensor(f"cc_dd{r}", [N, C + 1], f32).ap()
    d["ccr_dd"] = nc.dram_tensor(f"ccr_dd{r}", [N, C + 1], f32).ap()
    return d


def _body(nc, tc, d, w, rep, coll):
    """Emit one full pipeline. d = external IO handles, w = internal DRAM."""
    r = f"_{rep}"
    bb, bf = d["blob_bf"], d["blob_f32"]

    def _bf(key):
        lo, hi = BF_OFF[key]
        return bb[lo:hi]

    def _f32(key):
        lo, hi = F32_OFF[key]
        return bf[lo:hi]

    out_d = d["out"]
    rs_dd = w["rs_dd"]; ro_dd = w["ro_dd"]; cw_dd = w["cw_dd"]
    cc_dd = w["cc_dd"]; ccr_dd = w["ccr_dd"]

    with ExitStack() as top:
        top.enter_context(nc.allow_low_precision(
            "bf16/f32r staging throughout; all contractions accumulate fp32 in PSUM"))
        consts = top.enter_context(tc.tile_pool(name=f"consts{r}", bufs=1))
        late = top.enter_context(tc.tile_pool(name=f"late{r}", bufs=1))
        Tp = top.enter_context(tc.tile_pool(name=f"Tp{r}", bufs=1))

        T_sb = late.tile([128, PC, PQ], f32, tag="T")
        recip_pp = late.tile([128, PC], f32, tag="recip_pp")
        exp_pp = late.tile([128, PC], f32, tag="exp_pp")
        exp_bf = late.tile([128, PC], bf16, tag="exp_bf")
        zs_sb = late.tile([128, PC, Wd], bf16, tag="zs_z")
        zo_sb = late.tile([128, PC, Wd], bf16, tag="zo_z")
        AW1_sb = late.tile([128, AD], bf16, tag="AW1")
        AW2_sb = late.tile([16, AD], bf16, tag="AW2")
        ACl1_sb = late.tile([128, C], bf16, tag="ACl1")
        ACl2_sb = late.tile([16, C], bf16, tag="ACl2")
        cw_pp = late.tile([128, ACH], f32, tag="cw_pp")
        biasFrow = late.tile([1, C], bf16, tag="biasFrow")

        with tc.tile_pool(name=f"Ep{r}", bufs=1) as Ep, \
             tc.tile_pool(name=f"zstg{r}", bufs=2) as zstg, \
             tc.tile_pool(name=f"midp{r}", bufs=1) as midp:

            # ============ input DMAs, ordered by consumer urgency ========
            ga_sb = midp.tile([128, NS, N * HH], bf16, tag="ga")
            # first s-tile of ga lands first so pa can start early
            GA0, GA1 = BF_OFF["ga_all"]
            nc.sync.dma_start(ga_sb[:, 0, :],
                              bb[GA0:GA0 + 128 * N * HH].rearrange("(p f) -> p f", p=128))
            nc.scalar.dma_start(
                ga_sb[:, 1:NS, :],
                bb[GA0 + 128 * N * HH:GA1].rearrange("(t p f) -> p t f", p=128, f=N * HH))

            seqT_sb = midp.tile([128, HC, S], bf16, tag="seqT")
            nc.sync.dma_start(seqT_sb[:], _bf("seqT").rearrange("(c p s) -> p c s", p=128, s=S))
            Wc_sb = consts.tile([128, HC, Wd], bf16)
            nc.sync.dma_start(Wc_sb[:], _bf("Wc").rearrange("(c p w) -> p c w", p=128, w=Wd))
            sgT_sb = midp.tile([128, HC, NI * M], f32, tag="sgT")
            nc.sync.dma_start(sgT_sb[:], _f32("sgT").rearrange("(c p f) -> p c f", p=128, f=NI * M))
            Wso_sb = midp.tile([128, 2, HC, Wd], bf16, tag="Wso")
            nc.sync.dma_start(Wso_sb[:, 0], _bf("Ws").rearrange("(c p w) -> p c w", p=128, w=Wd))
            nc.sync.dma_start(Wso_sb[:, 1], _bf("Wo").rearrange("(c p w) -> p c w", p=128, w=Wd))

            Af_sb = midp.tile([128, HC, PQ], bf16, tag="Af")
            nc.sync.dma_start(Af_sb[:], _bf("Af").rearrange("(c p q) -> p c q", p=128, q=PQ))
            Wat_sb = midp.tile([128, HC, AD], bf16, tag="Wat")
            nc.sync.dma_start(Wat_sb[:], _bf("Wat").rearrange("(c p a) -> p c a", p=128, a=AD))
            clf_sb = midp.tile([128, HC, C], bf16, tag="clf")
            nc.sync.dma_start(clf_sb[:], _bf("clfW").rearrange("(c p o) -> p c o", p=128, o=C))
            eye_sb = consts.tile([128, 128], f32)
            nc.sync.dma_start(eye_sb[:], _f32("eye128").rearrange("(p q) -> p q", p=128))
            eyerow_sb = consts.tile([128, PC, NI], bf16)
            nc.sync.dma_start(eyerow_sb[:], _bf("eyerow").rearrange("(c p i) -> p c i", p=128, i=NI))
            eyecol_sb = consts.tile([128, PC, N], bf16)
            nc.sync.dma_start(eyecol_sb[:], _bf("eyecol").rearrange("(c p j) -> p c j", p=128, j=N))
            eyerowT_sb = consts.tile([NI, PC, 128], bf16)
            nc.sync.dma_start(eyerowT_sb[:], _bf("eyerowT").rearrange("(i c p) -> i c p", i=NI, p=128))
            eyecolT_sb = consts.tile([N, PC, 128], bf16)
            nc.sync.dma_start(eyecolT_sb[:], _bf("eyecolT").rearrange("(j c p) -> j c p", j=N, p=128))
            b_sb = midp.tile([128, HC], f32, tag="b_sb")
            nc.sync.dma_start(b_sb[:], _f32("bbl").rearrange("(c p) -> p c", p=128))
            v_sb = consts.tile([128, ACH], bf16)
            nc.sync.dma_start(v_sb[:], _bf("vat").rearrange("(c p) -> p c", p=128))
            clfb_row = midp.tile([1, C], f32, tag="clfb_row")
            nc.sync.dma_start(clfb_row[:], _f32("clfb").rearrange("(a c) -> a c", a=1))
            ones_row = consts.tile([1, 128], bf16)
            nc.vector.memset(ones_row[:], 1.0)

            # SWc rhs gets a ones column at index Wd -> ctx matmul also
            # produces the pa row-sums per pair partition.
            SWc_sb = midp.tile([128, NS, Wd + 1], bf16, tag="SWc")
            nc.vector.memset(SWc_sb[:, :, Wd:Wd + 1], 1.0)

            paT_sb = midp.tile([128, NS, P], bf16, tag="paT")

            with tc.tile_pool(name=f"psmain{r}", bufs=4, space="PSUM") as psmain:
                # rep logsumexp front (Act; its Pool reduce is emitted
                # inside the pa loop so it slots between the Pool tails)
                expg = midp.tile([128, HC * NI * M], f32, tag="expg")
                nc.scalar.activation(expg[:], sgT_sb[:].rearrange("p c f -> p (c f)"),
                                     mybir.ActivationFunctionType.Exp)
                rsum_m = midp.tile([128, HC * NI], f32, tag="rsum_m")
                repT_sb = midp.tile([128, HC, NI], bf16, tag="repT")
                egv = expg[:].rearrange("p (g m) -> p g m", m=M)
                rs2 = midp.tile([128, HC * NI * 2], f32, tag="rs2")
                rs2v = rs2[:].rearrange("p (g m) -> p g m", m=2)
                nc.gpsimd.tensor_add(rs2v, egv[:, :, 0:2], egv[:, :, 2:4])
                nc.gpsimd.tensor_add(rsum_m[:], rs2v[:, :, 0].squeeze(),
                                     rs2v[:, :, 1].squeeze())
                with nc.allow_low_precision("bf16 rep"):
                    nc.scalar.activation(
                        repT_sb[:].rearrange("p c i -> p (c i)"), rsum_m[:],
                        mybir.ActivationFunctionType.Ln)

                # ---- SW_c = seq @ W_c ----
                for t in range(NS):
                    sw_ps = psmain.tile([128, Wd + 1], f32, tag="ps")
                    for hc in range(HC):
                        lhsT = seqT_sb[:, hc, t * 128:(t + 1) * 128]
                        nc.tensor.matmul(sw_ps[:, 0:512], lhsT, Wc_sb[:, hc, 0:512],
                                         start=(hc == 0), stop=(hc == HC - 1))
                        nc.tensor.matmul(sw_ps[:, 512:768], lhsT, Wc_sb[:, hc, 512:768],
                                         start=(hc == 0), stop=(hc == HC - 1))
                    with nc.allow_low_precision("bf16 SWc"):
                        nc.scalar.copy(SWc_sb[:, t, 0:Wd], sw_ps[:, 0:Wd])

                # ---- rep @ Ws / Wo ----
                for wi, r_dd in ((0, rs_dd), (1, ro_dd)):
                    r_ps = psmain.tile([16, Wd + 1], f32, tag="ps")
                    for k in range(HC):
                        nc.tensor.matmul(r_ps[:, 0:512], repT_sb[:, k, :],
                                         Wso_sb[:, wi, k, 0:512], start=(k == 0), stop=(k == HC - 1))
                        nc.tensor.matmul(r_ps[:, 512:768], repT_sb[:, k, :],
                                         Wso_sb[:, wi, k, 512:768], start=(k == 0), stop=(k == HC - 1))
                    r_row = midp.tile([16, Wd], bf16, tag="r_row", bufs=2)
                    with nc.allow_low_precision("bf16 rs/ro staging"):
                        nc.scalar.copy(r_row[:], r_ps[:, 0:Wd])
                    nc.sync.dma_start(r_dd[:], r_row[:])

                # preload the Tanh act table during the idle window
                dummy_th = midp.tile([1, 1], f32, tag="dummy_th")
                nc.scalar.activation(dummy_th[:], r_row[0:1, 0:1],
                                     mybir.ActivationFunctionType.Tanh)

                # ---- weight pre-contractions (PE shadow work) ----
                # AW = Af @ W_attn [PQ, AD]; ACl = Af @ clf_W [PQ, C];
                # cw = (K b) @ W_attn; bf = (3 K b) @ clf_W
                bb_sb = midp.tile([128, HC], bf16, tag="bb_sb")
                bb3_sb = midp.tile([128, HC], bf16, tag="bb3_sb")
                with nc.allow_low_precision("bf16 bias staging"):
                    nc.scalar.mul(bb_sb[:], b_sb[:], float(KK))
                    nc.scalar.mul(bb3_sb[:], b_sb[:], 3.0 * KK)
                # fp32r matmul outputs must start on a PSUM bank
                # boundary, so the clf-side products live at column 512
                for lo, hi, aw_sb, acl_sb in ((0, 128, AW1_sb, ACl1_sb),
                                              (128, 144, AW2_sb, ACl2_sb)):
                    aw_ps = psmain.tile([hi - lo, Wd + 1], f32, tag="ps")
                    for wc in range(HC):
                        nc.tensor.matmul(aw_ps[:, 0:AD], Af_sb[:, wc, lo:hi],
                                         Wat_sb[:, wc, :],
                                         start=(wc == 0), stop=(wc == HC - 1))
                        nc.tensor.matmul(aw_ps[:, 512:512 + C], Af_sb[:, wc, lo:hi],
                                         clf_sb[:, wc, :],
                                         start=(wc == 0), stop=(wc == HC - 1))
                    with nc.allow_low_precision("bf16 AW/ACl"):
                        nc.scalar.copy(aw_sb[:], aw_ps[:, 0:AD])
                        nc.scalar.copy(acl_sb[:], aw_ps[:, 512:512 + C])
                cwb_ps = psmain.tile([1, Wd + 1], f32, tag="ps")
                for hc in range(HC):
                    nc.tensor.matmul(cwb_ps[:, 0:AD], bb_sb[:, hc:hc + 1],
                                     Wat_sb[:, hc, :],
                                     start=(hc == 0), stop=(hc == HC - 1))
                    nc.tensor.matmul(cwb_ps[:, 512:512 + C], bb3_sb[:, hc:hc + 1],
                                     clf_sb[:, hc, :],
                                     start=(hc == 0), stop=(hc == HC - 1))
                cw_row = midp.tile([1, AD], f32, tag="cw_row")
                nc.scalar.copy(cw_row[:], cwb_ps[:, 0:AD])
                nc.sync.dma_start(cw_dd.unsqueeze(0), cw_row[:])
                nc.sync.dma_start(cw_pp[:], cw_dd.rearrange("(c p) -> p c", c=ACH))
                with nc.allow_low_precision("bf16 bias"):
                    nc.vector.scalar_tensor_tensor(
                        biasFrow[:], cwb_ps[:, 512:512 + C], 1.0, clfb_row[:],
                        op0=mybir.AluOpType.mult, op1=mybir.AluOpType.add)

                # rs/ro replicated across the pair partitions
                rrep = {}
                for nm, r_dd in (("rs", rs_dd), ("ro", ro_dd)):
                    rr = midp.tile([128, PC, Wd], bf16, tag=f"{nm}_rep")
                    for pc in range(PC):
                        nc.sync.dma_start(
                            rr[:, pc, :],
                            r_dd[pc * 4:(pc + 1) * 4].unsqueeze(1).broadcast_to((4, N, Wd)))
                    rrep[nm] = rr

                # ---- pa products (DVE + Pool tail) + ctx matmuls chasing ----
                # i-side = first NI entities of ga (host rotates columns so the
                # core's own entities lead).
                ctxps = [psmain.tile([128, Wd + 1], f32, tag="ps", name=f"ctx{pc}{r}")
                         for pc in range(PC)]
                for t in range(NS):
                    E = Ep.tile([128, NI * N * HH], bf16, tag="E")
                    nc.vector.tensor_mul(
                        E[:].rearrange("p (i j h) -> p i j h", i=NI, j=N),
                        ga_sb[:, t, 0:NI * HH].rearrange("p (i h) -> p i h", i=NI)
                            .unsqueeze(2).broadcast_to((128, NI, N, HH)),
                        ga_sb[:, t, :].rearrange("p (j h) -> p j h", j=N)
                            .unsqueeze(1).broadcast_to((128, NI, N, HH)))
                    # pairwise tree sum over h=12 (2x bf16 on DVE; Pool
                    # takes the tails, plus r2 on some tiles for balance)
                    Ev = E[:].rearrange("p (ij h) -> p ij h", h=HH)
                    r1 = Ep.tile([128, P * 6], bf16, tag="ttA")
                    r1v = r1[:].rearrange("p (ij h) -> p ij h", h=6)
                    nc.vector.tensor_add(r1v, Ev[:, :, 0:6], Ev[:, :, 6:12])
                    r2 = Ep.tile([128, P * 3], bf16, tag="ttB", bufs=2)
                    r2v = r2[:].rearrange("p (ij h) -> p ij h", h=3)
                    r2eng = nc.gpsimd if t in (0, 2, 4, 6) else nc.vector
                    r2eng.tensor_add(r2v, r1v[:, :, 0:3], r1v[:, :, 3:6])
                    teng = nc.vector if t == NS - 1 else nc.gpsimd
                    r3 = Ep.tile([128, P], bf16, tag="r3", bufs=2)
                    teng.tensor_add(r3[:], r2v[:, :, 0].squeeze(), r2v[:, :, 1].squeeze())
                    with nc.allow_low_precision("bf16 pa tree"):
                        teng.tensor_add(paT_sb[:, t, :], r3[:], r2v[:, :, 2].squeeze())
                    for pc in range(PC):
                        lhsT = paT_sb[:, t, pc * 128:(pc + 1) * 128]
                        nc.tensor.matmul(ctxps[pc][:, 0:512], lhsT, SWc_sb[:, t, 0:512],
                                         start=(t == 0), stop=(t == NS - 1))
                        nc.tensor.matmul(ctxps[pc][:, 512:Wd + 1], lhsT, SWc_sb[:, t, 512:Wd + 1],
                                         start=(t == 0), stop=(t == NS - 1))

                # ---- z = tanh(ctx/rsum + repW): Act pre-scales ctx by 1/rsum
                # to bf16, DVE does only a 2x-mode bf16 add, Act applies tanh.
                # reciprocal emitted per chunk so z0 never waits on ctx3 ----
                def emit_z(pc):
                    nc.vector.reciprocal(recip_pp[:, pc:pc + 1], ctxps[pc][:, Wd:Wd + 1])
                    zsc = zstg.tile([128, Wd], bf16, tag="zsc")
                    with nc.allow_low_precision("bf16 ctx scale"):
                        nc.scalar.mul(zsc[:], ctxps[pc][:, 0:Wd], recip_pp[:, pc:pc + 1])
                    for z_sb, rrk in ((zs_sb, "rs"), (zo_sb, "ro")):
                        zpre = zstg.tile([128, Wd], bf16, tag="zpre2")
                        with nc.allow_low_precision("bf16 z"):
                            nc.vector.tensor_add(zpre[:], zsc[:], rrep[rrk][:, pc, :])
                        nc.scalar.activation(z_sb[:, pc, :], zpre[:],
                                             mybir.ActivationFunctionType.Tanh)

                def emit_tprod(pc):
                    ET = Tp.tile([128, PQ * KK], bf16, tag="E2")
                    nc.vector.tensor_mul(
                        ET[:].rearrange("p (a q k) -> p a q k", a=DD, q=DD),
                        zs_sb[:, pc, :].rearrange("p (a k) -> p a k", a=DD)
                            .unsqueeze(2).broadcast_to((128, DD, DD, KK)),
                        zo_sb[:, pc, :].rearrange("p (q k) -> p q k", q=DD)
                            .unsqueeze(1).broadcast_to((128, DD, DD, KK)))
                    cur = ET[:].rearrange("p (pq k) -> p pq k", k=KK)
                    w = KK
                    ab = 0
                    # last chunk: keep the tail on DVE (it idles after the
                    # multiplies; queued-up Pool would sit on the critical path)
                    tail_eng = nc.vector if pc == PC - 1 else nc.gpsimd
                    while w > 2:
                        w //= 2
                        eng = nc.vector if w >= 16 else tail_eng
                        nxt = Tp.tile([128, PQ * w], bf16, tag=f"tt{'AB'[ab % 2]}")
                        ab += 1
                        nxtv = nxt[:].rearrange("p (pq k) -> p pq k", k=w)
                        eng.tensor_add(nxtv, cur[:, :, 0:w], cur[:, :, w:2 * w])
                        cur = nxtv
                    with nc.allow_low_precision("bf16 T tree"):
                        tail_eng.tensor_add(T_sb[:, pc, :], cur[:, :, 0].squeeze(),
                                            cur[:, :, 1].squeeze())

                emit_z(0)
                emit_z(1)
                emit_tprod(0)
                emit_z(2)
                emit_tprod(1)
                emit_z(3)
                emit_tprod(2)
                emit_tprod(3)

        # ============ post-T chase: everything bf16, pair-major ============
        with tc.tile_pool(name=f"pstp{r}", bufs=1, space="PSUM") as pstp, \
             tc.tile_pool(name=f"psmm{r}", bufs=3, space="PSUM") as psmm, \
             tc.tile_pool(name=f"psacc{r}", bufs=1, space="PSUM") as psacc:

            # one accumulation group per PSUM bank (start zeroes the
            # whole 2KB region): row/col context contractions and the exp
            # row/col sums each get their own bank with a plain
            # start-at-pc0 / stop-at-pc3 group
            mtr = psacc.tile([NI, C], f32, tag="mtr")
            mtc = psacc.tile([N, C], f32, tag="mtc")
            rsum_ps = psacc.tile([NI, 1], f32, tag="rsum")
            csum_ps = psacc.tile([N, 1], f32, tag="csum")

            TT1_sb = late.tile([128, P], bf16, tag="TT1")
            TT2_sb = late.tile([16, P], bf16, tag="TT2")
            gclf_sb = late.tile([128, PC, C], bf16, tag="gclf")
            th_sb = late.tile([128, ACH, P], bf16, tag="th")
            Mrc_sb = late.tile([128, PC, NI + N], bf16, tag="Mrc")

            def emit_tp(pc):
                tp1 = pstp.tile([128, 128], f32, tag="tp")
                nc.tensor.transpose(tp1[:], T_sb[:, pc, 0:128], eye_sb[:])
                with nc.allow_low_precision("bf16 TT"):
                    nc.scalar.copy(TT1_sb[:, pc * 128:(pc + 1) * 128], tp1[:])
                tp2 = pstp.tile([16, 128], f32, tag="tp")
                nc.tensor.transpose(tp2[:], T_sb[:, pc, 128:144], eye_sb[:])
                with nc.allow_low_precision("bf16 TT"):
                    nc.scalar.copy(TT2_sb[:, pc * 128:(pc + 1) * 128], tp2[:])

            def emit_gclf(pc):
                # per-pair classifier logits of g (sans contexts/bias),
                # both into the output accumulator and as a bf16 copy for
                # the row/col context contraction
                sl = slice(pc * 128, (pc + 1) * 128)
                gp = psmm.tile([128, 512], f32, tag="mm")
                nc.tensor.matmul(gp[:, 0:C], TT1_sb[:, sl], ACl1_sb[:],
                                 start=True, stop=False)
                nc.tensor.matmul(gp[:, 0:C], TT2_sb[:, sl], ACl2_sb[:],
                                 start=False, stop=True)
                with nc.allow_low_precision("bf16 gclf"):
                    nc.scalar.copy(gclf_sb[:, pc, :], gp[:, 0:C])

            def emit_th_e(pcs):
                # th = tanh(T @ AW + cw), per pair chunk (128-wide bf16
                # matmuls run at full rate), then e + exp per chunk
                for pc in pcs:
                    sl = slice(pc * 128, (pc + 1) * 128)
                    for ac in range(ACH):
                        ep_ps = psmm.tile([128, 512], f32, tag="mm")
                        nc.tensor.matmul(ep_ps[:, 0:128], AW1_sb[:, ac * 128:(ac + 1) * 128],
                                         TT1_sb[:, sl], start=True, stop=False)
                        nc.tensor.matmul(ep_ps[:, 0:128], AW2_sb[:, ac * 128:(ac + 1) * 128],
                                         TT2_sb[:, sl], start=False, stop=True)
                        with nc.allow_low_precision("bf16 th"):
                            nc.scalar.activation(th_sb[:, ac, sl], ep_ps[:, 0:128],
                                                 mybir.ActivationFunctionType.Tanh,
                                                 bias=cw_pp[:, ac:ac + 1])
                    e_ps = psmm.tile([128, 512], f32, tag="mm")
                    for ac in range(ACH):
                        nc.tensor.matmul(e_ps[:, 0:1],
                                         th_sb[:, ac, sl],
                                         v_sb[:, ac:ac + 1],
                                         start=(ac == 0), stop=(ac == ACH - 1))
                    nc.scalar.activation(exp_pp[:, pc:pc + 1], e_ps[:, 0:1],
                                         mybir.ActivationFunctionType.Exp)
                    with nc.allow_low_precision("bf16 exp"):
                        nc.gpsimd.tensor_copy(exp_bf[:, pc:pc + 1], exp_pp[:, pc:pc + 1])

            def emit_sel(pcs):
                """selectors, exp row/col sums, Mrc^T gclf accumulation."""
                for pc in pcs:
                    # per-partition scale-by-exp runs on the scalar engine
                    with nc.allow_low_precision("bf16 selectors"):
                        nc.scalar.mul(Mrc_sb[:, pc, 0:NI], eyerow_sb[:, pc, :],
                                      exp_pp[:, pc:pc + 1])
                        nc.scalar.mul(Mrc_sb[:, pc, NI:NI + N], eyecol_sb[:, pc, :],
                                      exp_pp[:, pc:pc + 1])
                    nc.tensor.matmul(rsum_ps[:], eyerow_sb[:, pc, :],
                                     exp_bf[:, pc:pc + 1],
                                     start=(pc == 0), stop=(pc == PC - 1))
                    nc.tensor.matmul(csum_ps[:], eyecol_sb[:, pc, :],
                                     exp_bf[:, pc:pc + 1],
                                     start=(pc == 0), stop=(pc == PC - 1))
                    nc.tensor.matmul(mtr[:], Mrc_sb[:, pc, 0:NI],
                                     gclf_sb[:, pc, :],
                                     start=(pc == 0), stop=(pc == PC - 1))
                    nc.tensor.matmul(mtc[:], Mrc_sb[:, pc, NI:NI + N],
                                     gclf_sb[:, pc, :],
                                     start=(pc == 0), stop=(pc == PC - 1))

            emit_tp(0)
            emit_tp(1)
            emit_gclf(0)
            emit_gclf(1)
            emit_th_e((0, 1))
            emit_sel((0, 1))
            emit_tp(2)
            emit_tp(3)
            emit_gclf(2)
            emit_gclf(3)
            emit_th_e((2, 3))
            emit_sel((2, 3))

            # ---- local sums out; one combined pair AllReduce ----
            cs_col = late.tile([N, 1], f32, tag="cs_col")
            nc.scalar.copy(cs_col[:], csum_ps[:])
            nc.sync.dma_start(cc_dd[0:N, C:C + 1], cs_col[:])
            ccu_sb = late.tile([N, C], f32, tag="ccu")
            nc.scalar.copy(ccu_sb[:], mtc[:])
            nc.sync.dma_start(cc_dd[0:N, 0:C], ccu_sb[:])
            if coll:
                nc.gpsimd.collective_compute("AllReduce", mybir.AluOpType.add,
                                             replica_groups=GROUPS, ins=[cc_dd[:]],
                                             outs=[ccr_dd[:]])
            else:
                nc.sync.dma_start(ccr_dd[0:N, C:C + 1], cs_col[:])
                nc.sync.dma_start(ccr_dd[0:N, 0:C], ccu_sb[:])
            # ---- local row path (overlaps the collective) ----
            rrec_col = late.tile([NI, 1], f32, tag="rrec_col")
            nc.vector.reciprocal(rrec_col[:], rsum_ps[:])
            rcn_r = late.tile([NI, C], bf16, tag="rcn_r")
            with nc.allow_low_precision("bf16 contexts"):
                nc.vector.tensor_scalar_mul(rcn_r[:], mtr[:], rrec_col[:])

            # ---- column path after the collective lands (one DMA) ----
            ccg_sb = late.tile([N, C + 1], f32, tag="ccg")
            nc.sync.dma_start(ccg_sb[:], ccr_dd[:])
            crec_col = late.tile([N, 1], f32, tag="crec_col")
            nc.vector.reciprocal(crec_col[:], ccg_sb[:, C:C + 1])
            rcn_c = late.tile([N, C], bf16, tag="rcn_c")
            with nc.allow_low_precision("bf16 contexts"):
                nc.vector.tensor_scalar_mul(rcn_c[:], ccg_sb[:, 0:C], crec_col[:])
            out_sb = late.tile([128, PC, C], f32, tag="out_sb")
            for pc in range(PC):
                cx = psmm.tile([128, 512], f32, tag="mm")
                nc.tensor.matmul(cx[:, 0:C], eyerowT_sb[:, pc, :], rcn_r[:],
                                 start=True, stop=False)
                nc.tensor.matmul(cx[:, 0:C], ones_row[:], biasFrow[:],
                                 start=False, stop=False)
                nc.tensor.matmul(cx[:, 0:C], eyecolT_sb[:, pc, :], rcn_c[:],
                                 start=False, stop=True)
                nc.vector.tensor_add(out_sb[:, pc, :], cx[:, 0:C], gclf_sb[:, pc, :])
                if pc == 1:
                    nc.sync.dma_start(
                        out_d.rearrange("(c p) o -> p c o", p=128)[:, 0:2, :],
                        out_sb[:, 0:2, :])
            nc.sync.dma_start(out_d.rearrange("(c p) o -> p c o", p=128)[:, 2:PC, :],
                              out_sb[:, 2:PC, :])


def build_program(coll=True, reps=1):
    nc = bacc.Bacc("TRN2", target_bir_lowering=False, debug=False, num_devices=8)
    d = _make_io(nc)
    internals = [_make_internal(nc, f"_{rep}") for rep in range(reps)]
    use_coll = coll and reps == 1
    with tile.TileContext(nc) as tc:
        for rep in range(reps):
            _body(nc, tc, d, internals[rep], rep, use_coll)
    nc.compile()
    return nc


def host_shard(inputs):
    seq_all = np.ascontiguousarray(inputs["sequence_output"], dtype=np.float32)
    att_all = np.ascontiguousarray(inputs["attention"], dtype=np.float32)
    ep_all = np.asarray(inputs["entity_pos"])
    W_s = np.ascontiguousarray(inputs["W_s"], dtype=np.float32)
    W_o = np.ascontiguousarray(inputs["W_o"], dtype=np.float32)
    W_c = np.ascontiguousarray(inputs["W_c"], dtype=np.float32)
    A_bl = np.ascontiguousarray(inputs["A_bl"], dtype=np.float32)
    b_bl = np.ascontiguousarray(inputs["b_bl"], dtype=np.float32)
    W_attn = np.ascontiguousarray(inputs["W_attn"], dtype=np.float32)
    v_attn = np.ascontiguousarray(inputs["v_attn"], dtype=np.float32)
    clf_W = np.ascontiguousarray(inputs["clf_W"], dtype=np.float32)
    clf_b = np.ascontiguousarray(inputs["clf_b"], dtype=np.float32)

    Af = np.ascontiguousarray(A_bl.reshape(Wd, PQ))               # [w, (p,q)]
    eye128 = np.eye(128, dtype=np.float32)
    eyerow = np.repeat(np.eye(NI, dtype=np.float32), N, axis=0)   # [P, NI]

    shared = dict(Ws=W_s, Wo=W_o, Wc=W_c, Af=Af, bbl=b_bl, Wat=W_attn,
                  vat=v_attn, clfW=clf_W, clfb=clf_b, eye128=eye128,
                  eyerow=eyerow, eyerowT=np.ascontiguousarray(eyerow.T))

    def pack(pieces, vals, dtype):
        parts = [np.ascontiguousarray(vals[k], dtype=np.float32).ravel() for k, _ in pieces]
        for (k, n), p in zip(pieces, parts):
            assert p.size == n, (k, p.size, n)
        return np.concatenate(parts).astype(dtype)

    in_maps = []
    for doc in range(B):
        seq = seq_all[doc]
        ep = ep_all[doc].astype(np.int64)
        ga = att_all[doc][:, ep.reshape(-1), :].reshape(HH, N, M, S).sum(axis=2)  # [h, n, s]
        ga_T = np.ascontiguousarray(ga.transpose(2, 1, 0).reshape(S, N * HH))     # [s, (n,h)]
        seq_gT = np.ascontiguousarray(seq[ep.reshape(-1), :].T)                   # [H, N*M]
        for half in range(2):
            i0 = half * NI
            # rotate entity columns so this core's own 16 entities lead;
            # order[j_local] = global entity index of local column j_local
            order = np.concatenate([np.arange(i0, i0 + NI), np.arange(0, i0),
                                    np.arange(i0 + NI, N)]).astype(np.int64)
            gav = ga_T.reshape(S, N, HH)[:, order, :].reshape(S, N * HH)
            # eyecol maps local pair lane -> GLOBAL entity column (collective
            # alignment); eyecolT scatters global column context back to lanes
            eyecol = np.tile(np.eye(N, dtype=np.float32)[order], (NI, 1))     # [P, N]
            vals = dict(shared)
            vals["ga_all"] = gav
            vals["seqT"] = seq.T
            vals["sgT"] = seq_gT[:, i0 * M:(i0 + NI) * M]
            vals["eyecol"] = eyecol
            vals["eyecolT"] = eyecol.T
            in_maps.append({
                "blob_bf": pack(BF_PIECES, vals, ml_dtypes.bfloat16),
                "blob_f32": pack(F32_PIECES, vals, np.float32),
            })
    return in_maps


def assemble(results):
    out = np.empty((B, N, N, C), np.float32)
    for core in range(2 * B):
        doc, half = divmod(core, 2)
        i0 = half * NI
        order = np.concatenate([np.arange(i0, i0 + NI), np.arange(0, i0),
                                np.arange(i0 + NI, N)])
        inv = np.argsort(order)
        block = results[core]["out"].reshape(NI, N, C)
        out[doc, i0:i0 + NI] = block[:, inv, :]
    return out


def kernel(**inputs):
    nc = _CACHE.get("nc")
    if nc is None:
        nc = build_program()
        _CACHE["nc"] = nc
    in_maps = host_shard(inputs)
    res = run_bass_kernel_spmd(nc, in_maps, list(range(2 * B)))
    _CACHE["last_res"] = res
    return assemble(res.results)
